# revision 49
# baseline (speedup 1.0000x reference)
"""Trainium2 Bass kernel for nn_MixedMlp (soft-mixture MoE MLP).

Math (per batch row b):
    cn = LayerNorm(c); x = [z, cn]
    coeff = softmax(gateMLP(x))                       # [E]
    l0 = elu(sum_e coeff_e (x @ w0_e + b0_e))
    l1 = elu(sum_e coeff_e ([z, l0] @ w1_e + b1_e))
    out = sum_e coeff_e ([z, l1] @ w2_e + b2_e)

Kernel strategy (8 cores, data-parallel over B=8192), v2 pipelined:
  * Activations feature-major ([features, batch]); every layer is one
    PSUM-accumulated GEMM contracting K = E*in over coeff-scaled inputs.
  * Batch split into 4 chunks of 256 rows; LN -> gate -> coeff broadcast
    -> expert layers -> store software-pipelined across chunks.
  * c ships both batch-major fp32 (for bn_stats) and feature-major fp16
    (cT); LN applies as cn = (cT - m_bc) * rstd_bc with mean/rstd
    DMA-broadcast along partitions — no PE transposes, no scalar affine.
    gamma/beta folded into w0c/g0c host-side.
  * Each layer's two 128-row output halves accumulate into ONE [128,512]
    PSUM bank tile, so elu exp/relu/combine run as single 512-wide ops.
  * Scaled inputs built as one DVE tensor_mul per (layer, feat-tile,
    chunk): stride-0 repeat of the activation x expert-major coeff
    broadcast (8 experts per instruction). GpSimd only does the small
    z-scaling; it is ~4x slower than DVE per element.
  * ELU: s = elu(x)+1 = relu(x) + min(exp(x),1); -1 folded into next
    bias host-side. exp/relu on Scalar, combine on DVE.
  * Layer 2 stacks per-expert outputs [(e,o), b], mixes by expanded
    coeff, selector-matmuls into [128,32] PSUM, DMA'd straight to DRAM.
  * coeff broadcasts via DMA replication from a DRAM staging buffer,
    per-chunk, spread across sync/gpsimd/scalar queues.
"""

import numpy as np
from contextlib import ExitStack

import concourse.bass as bass
import concourse.bacc as bacc
import concourse.tile as tile
import concourse.mybir as mybir
from concourse import bass_utils
from concourse.bass import AP

F32 = mybir.dt.float32
F32R = mybir.dt.float16  # full fp16 datapath
AF = mybir.ActivationFunctionType
OP = mybir.AluOpType

N_CORES = 8
B = 8192
R = B // N_CORES          # rows per core = 1024
LATENT, CIN, HID, ACTD, E, GH = 32, 128, 256, 16, 8, 128
IN0, INTER = LATENT + CIN, HID + LATENT
LN_EPS = 1e-5
BT = 256                  # batch chunk (pipeline granularity)
NCH = R // BT             # 4 chunks per core
NLC = R // 128            # 8 ln-chunks per core

# packed-weight column offsets (two packs: gate-critical, expert bulk)
_GCOLS = [("g0z", 128), ("g0c", 128), ("g1w", 128), ("g2w", 8),
          ("b01", 512), ("on8", 1), ("onr", 8), ("i16", 128)]
_WCOLS = [("w0z", 512), ("w0c", 2048), ("w1z", 512), ("w1h", 4096),
          ("w2s", 384), ("s2", 16)]
_GOFF, _WOFF = {}, {}
_o = 0
for _n, _c in _GCOLS:
    _GOFF[_n] = _o
    _o += _c
NGATE = _o
_o = 0
for _n, _c in _WCOLS:
    _WOFF[_n] = _o
    _o += _c
NWALL = _o

_CACHE = {}


def _rep_ap(t_ap, nrep, width):
    """AP reading t_ap[:, 0:width] repeated nrep times along a stride-0
    middle dim: shape [128, nrep, width]."""
    part = list(t_ap.ap[0])
    return AP(t_ap.tensor, t_ap.offset, [part, [0, nrep], [1, width]])


def _build_program():
    nc = bacc.Bacc("TRN2", target_bir_lowering=False, debug=False,
                   num_devices=N_CORES)

    zr_d = nc.dram_tensor("zrep", [128, R], F32R, kind="ExternalInput").ap()
    c_d = nc.dram_tensor("cperm", [128, NLC * CIN], F32, kind="ExternalInput").ap()
    wg_d = nc.dram_tensor("wgate", [128, NGATE], F32R, kind="ExternalInput").ap()
    wall_d = nc.dram_tensor("wall", [128, NWALL], F32R, kind="ExternalInput").ap()
    ck_d = nc.dram_tensor("consts", [128, 7], F32, kind="ExternalInput").ap()
    out_d = nc.dram_tensor("out", [R, ACTD], F32, kind="ExternalOutput").ap()

    with tile.TileContext(nc) as tc, ExitStack() as ctx:
        wp = ctx.enter_context(tc.tile_pool(name="wp", bufs=1))       # weights
        big = ctx.enter_context(tc.tile_pool(name="big", bufs=1))     # persistent
        sp = ctx.enter_context(tc.tile_pool(name="sp", bufs=4))       # small temps
        er = ctx.enter_context(tc.tile_pool(name="er", bufs=4))       # elu temps
        hp = ctx.enter_context(tc.tile_pool(name="hp", bufs=4))       # gate hiddens
        scp = ctx.enter_context(tc.tile_pool(name="scp", bufs=2))     # scaled inputs
        s0p = ctx.enter_context(tc.tile_pool(name="s0p", bufs=2))     # layer outs
        cbp = ctx.enter_context(tc.tile_pool(name="cbp", bufs=4))     # coeff bcasts
        pm = ctx.enter_context(tc.tile_pool(name="pm", bufs=2, space="PSUM"))
        pm2 = ctx.enter_context(tc.tile_pool(name="pm2", bufs=4, space="PSUM"))
        pt = ctx.enter_context(tc.tile_pool(name="pt", bufs=1, space="PSUM"))
        po = ctx.enter_context(tc.tile_pool(name="po", bufs=1, space="PSUM"))
        dstage = ctx.enter_context(tc.tile_pool(name="dstage", bufs=1, space="DRAM"))

        # ---------------- bulk loads ----------------
        # cperm: partition p holds rows 8p..8p+8 (stats layout).  ctr/zrep:
        # feature-major, on-chip batch order i = 128*r + p <-> row 8p + r.
        # priority order: LN inputs + gate weights first; the big expert
        # weight pack (wall) is deferred until after the gate emission so
        # its 2MB doesn't clog the DMA engines ahead of the critical path.
        ctall = big.tile([128, NLC * CIN], F32)
        ckt = wp.tile([128, 7], F32)
        nc.sync.dma_start(ckt[:], ck_d[:])
        for hh in range(2):
            cs = slice(2 * CIN * hh, 2 * CIN * (hh + 1))
            nc.sync.dma_start(ctall[:, cs], c_d[:, cs])
        wgate = wp.tile([128, NGATE], F32R)
        nc.scalar.dma_start(wgate[:], wg_d[:], max_dma_last_dim=4096)
        zrep = big.tile([128, R], F32R)
        nc.scalar.dma_start(zrep[:], zr_d[:])
        for hh in range(2, 4):
            cs = slice(2 * CIN * hh, 2 * CIN * (hh + 1))
            nc.sync.dma_start(ctall[:, cs], c_d[:, cs])
        wall = wp.tile([128, NWALL], F32R)

        def emit_wall_load():
            h2 = NWALL // 2
            nc.scalar.dma_start(wall[:, 0:h2], wall_d[:, 0:h2],
                                max_dma_last_dim=4096)
            nc.scalar.dma_start(wall[:, h2:], wall_d[:, h2:],
                                max_dma_last_dim=4096)

        def wsl(name, p0, pn, c0, cn_):
            if name in _GOFF:
                o = _GOFF[name]
                return wgate[p0:p0 + pn, o + c0:o + c0 + cn_]
            o = _WOFF[name]
            return wall[p0:p0 + pn, o + c0:o + c0 + cn_]
        epsc = ckt[:, 2:3]
        g0b, g1b, b2c = ckt[:, 3:4], ckt[:, 4:5], ckt[:, 5:6]
        g2b = ckt[0:8, 6:7]

        # ---------------- persistent tiles ----------------
        cnT = big.tile([128, R], F32R)     # LayerNormed c (affine folded)
        zs2 = big.tile([128, 2 * R], F32R)  # coeff-scaled z, 2 quad-groups
        eL = big.tile([8, R], F32R)        # exp(gate logits)
        coeffN = big.tile([8, R], F32R)    # softmax coeffs
        mv8 = big.tile([128, 2 * NLC], F32)
        rstd8 = big.tile([128, NLC], F32)

        cstage = dstage.tile([8, R], F32R)
        ctens = cstage.tensor

        # ------- LN stats in two batches; rstd via Ln+Exp each -------
        lnv8 = big.tile([128, NLC], F32)

        def emit_stats(b):
            for j in range(4 * b, 4 * b + 4):
                ct = ctall[:, CIN * j:CIN * (j + 1)]
                stats = sp.tile([128, 6], F32, tag="st")
                nc.vector.bn_stats(stats[:], ct[:])
                nc.vector.bn_aggr(mv8[:, 2 * j:2 * j + 2], stats[:])
            var4 = AP(mv8[:].tensor, mv8[:].offset + 8 * b + 1,
                      [list(mv8[:].ap[0]), [2, 4]])
            nc.scalar.activation(lnv8[:, 4 * b:4 * b + 4], var4, AF.Ln,
                                 bias=epsc[:])
            nc.scalar.activation(rstd8[:, 4 * b:4 * b + 4],
                                 lnv8[:, 4 * b:4 * b + 4], AF.Exp, scale=-0.5)

        # ---------------- per-chunk emitters ----------------
        # LN tail on DVE: scale via tensor_scalar, evac via copy.
        def emit_cn(k):
            for j in (2 * k, 2 * k + 1):
                js = slice(CIN * j, CIN * (j + 1))
                y = sp.tile([128, CIN], F32R, tag="y")
                nc.vector.tensor_scalar(y[:], ctall[:, js],
                                        mv8[:, 2 * j:2 * j + 1],
                                        rstd8[:, j:j + 1],
                                        OP.subtract, OP.mult)
                yT = pt.tile([128, CIN], F32R, tag="tp")
                nc.tensor.transpose(yT[:], y[:], wsl("i16", 0, 128, 0, 128))
                nc.vector.tensor_copy(cnT[:, js], yT[:])

        # gate at 256-wide, 4 chains staged so they pipeline each other's
        # cross-engine latency
        _g = [dict() for _ in range(NCH)]

        def gate_a(k):
            bs = slice(BT * k, BT * (k + 1))
            pre0 = pm2.tile([128, BT], F32, tag="mm2", name=f"g0_{k}")
            nc.tensor.matmul(pre0[:], wsl("g0z", 0, 32, 0, 128), zrep[0:32, bs],
                             start=True, stop=False)
            nc.tensor.matmul(pre0[:], wsl("g0c", 0, 128, 0, 128), cnT[:, bs],
                             start=False, stop=True)
            e0 = er.tile([128, BT], F32R, tag="eg", name=f"e0_{k}")
            nc.scalar.activation(e0[:], pre0[:], AF.Exp, bias=g0b[:])
            r0 = er.tile([128, BT], F32R, tag="rg", name=f"r0_{k}")
            nc.vector.tensor_scalar(r0[:], pre0[:], g0b[:], 0.0,
                                    OP.add, OP.max)
            h0 = hp.tile([128, BT], F32R, tag="h0", name=f"h0_{k}")
            nc.vector.scalar_tensor_tensor(h0[:], e0[:], 1.0, r0[:],
                                           OP.min, OP.add)
            _g[k]["h0"] = h0

        def gate_b(k):
            pre1 = pm2.tile([128, BT], F32, tag="mm2", name=f"g1_{k}")
            nc.tensor.matmul(pre1[:], wsl("g1w", 0, 128, 0, 128),
                             _g[k]["h0"][:], start=True, stop=True)
            e1 = er.tile([128, BT], F32R, tag="eg", name=f"e1_{k}")
            nc.scalar.activation(e1[:], pre1[:], AF.Exp, bias=g1b[:])
            r1 = er.tile([128, BT], F32R, tag="rg", name=f"r1_{k}")
            nc.vector.tensor_scalar(r1[:], pre1[:], g1b[:], 0.0,
                                    OP.add, OP.max)
            h1 = hp.tile([128, BT], F32R, tag="h1", name=f"h1_{k}")
            nc.vector.scalar_tensor_tensor(h1[:], e1[:], 1.0, r1[:],
                                           OP.min, OP.add)
            _g[k]["h1"] = h1

        def gate_c(k):
            bs = slice(BT * k, BT * (k + 1))
            pre2 = pm2.tile([8, BT], F32, tag="mm2", name=f"g2_{k}")
            nc.tensor.matmul(pre2[:], wsl("g2w", 0, 128, 0, 8),
                             _g[k]["h1"][:], start=True, stop=True)
            nc.scalar.activation(eL[:, bs], pre2[:], AF.Exp, bias=g2b[:])
            sume = pm2.tile([1, BT], F32, tag="mm2", name=f"gs_{k}")
            nc.tensor.matmul(sume[:], wsl("on8", 0, 8, 0, 1), eL[:, bs],
                             start=True, stop=True)
            rsum = sp.tile([1, BT], F32, tag="rsm", name=f"rsum{k}")
            nc.vector.reciprocal_approx_fast(rsum[:], sume[:])
            rsr = sp.tile([1, BT], F32R, tag="rsr", name=f"rsr{k}")
            nc.vector.tensor_copy(rsr[:], rsum[:])
            _g[k]["rsr"] = rsr

        def gate_d(k):
            bs = slice(BT * k, BT * (k + 1))
            rbc = pm2.tile([8, BT], F32, tag="mm2", name=f"gr_{k}")
            nc.tensor.matmul(rbc[:], wsl("onr", 0, 1, 0, 8), _g[k]["rsr"][:],
                             start=True, stop=True)
            nc.vector.tensor_mul(coeffN[:, bs], eL[:, bs], rbc[:])

        def emit_bcast(k):
            bs = slice(BT * k, BT * (k + 1))
            o = BT * k
            nc.sync.dma_start(cstage[:, bs], coeffN[:, bs])
            cb = cbp.tile([128, E * BT], F32R, tag="cb", name=f"cb{k}")
            for eh, q in ((0, nc.sync), (1, nc.gpsimd), (2, nc.scalar),
                          (3, nc.sync)):
                q.dma_start(
                    AP(cb.tensor, cb[:].offset + 2 * eh * BT,
                       [[E * BT, 128], [BT, 2], [1, BT]]),
                    AP(ctens, 2 * eh * R + o, [[0, 128], [R, 2], [1, BT]]))
            cbz = cbp.tile([128, 2 * BT], F32R, tag="cbz", name=f"cbz{k}")
            for q in range(2):
                nc.sync.dma_start(
                    cbz[:, BT * q:BT * (q + 1)],
                    AP(ctens, 4 * q * R + o, [[R, 4], [0, 32], [1, BT]]))
            cbe = cbp.tile([128, BT], F32R, tag="cbe", name=f"cbe{k}")
            nc.gpsimd.dma_start(
                cbe[:], AP(ctens, o, [[R, 8], [0, 16], [1, BT]]))
            # coeff-scaled z (both quad groups in one gpsimd op)
            dst = AP(zs2.tensor, zs2[:].offset + o,
                     [[2 * R, 128], [R, 2], [1, BT]])
            src0 = AP(zrep.tensor, zrep[:].offset + o,
                      [[R, 128], [0, 2], [1, BT]])
            nc.gpsimd.tensor_mul(dst, src0, cbz[:])
            return cb, cbe

        def elu_combine(ps, dst):
            ee = er.tile([128, 2 * BT], F32R, tag="e")
            nc.scalar.activation(ee[:], ps[:], AF.Exp)
            rr = er.tile([128, 2 * BT], F32R, tag="r")
            nc.scalar.activation(rr[:], ps[:], AF.Relu)
            nc.vector.scalar_tensor_tensor(dst[:], ee[:], 1.0, rr[:],
                                           OP.min, OP.add)

        def emit_l0(k, cb):
            bs = slice(BT * k, BT * (k + 1))
            # one [128,512] bank tile for both output halves: start=True only
            # on the very first matmul (it bank-clears); later slices
            # auto-zero on first write via the pending-zero region.
            # z-part matmuls go LAST so the gpsimd-built zs has time to land.
            ps = pm.tile([128, 2 * BT], F32, tag="mm", name=f"ps0_{k}")
            for mt in range(2):
                nc.tensor.matmul(ps[:, BT * mt:BT * (mt + 1)],
                                 wsl("b01", 0, 8, 128 * mt, 128),
                                 coeffN[:, bs], start=(mt == 0), stop=False,
                                 skip_group_check=True)
            sc0 = scp.tile([128, E * BT], F32R, tag="sc0", name=f"sc0_{k}")
            nc.vector.tensor_mul(sc0[:], _rep_ap(cnT[:, bs], E, BT), cb[:])
            for e in range(E):
                for mt in range(2):
                    nc.tensor.matmul(ps[:, BT * mt:BT * (mt + 1)],
                                     wsl("w0c", 0, 128, 256 * e + 128 * mt, 128),
                                     sc0[:, BT * e:BT * (e + 1)],
                                     start=False, stop=False,
                                     skip_group_check=True)
            for mt in range(2):
                for kt in range(2):
                    nc.tensor.matmul(ps[:, BT * mt:BT * (mt + 1)],
                                     wsl("w0z", 0, 128, 256 * kt + 128 * mt, 128),
                                     zs2[:, R * kt + BT * k:R * kt + BT * (k + 1)],
                                     start=False,
                                     stop=(mt == 1 and kt == 1),
                                     skip_group_check=True)
            s0 = s0p.tile([128, 2 * BT], F32R, tag="s0", name=f"s0_{k}")
            elu_combine(ps, s0)
            return s0

        def emit_l12(k, s0, cb, cbe):
            bs = slice(BT * k, BT * (k + 1))
            ps = pm.tile([128, 2 * BT], F32, tag="mm", name=f"ps1_{k}")
            for mt in range(2):
                nc.tensor.matmul(ps[:, BT * mt:BT * (mt + 1)],
                                 wsl("b01", 0, 8, 256 + 128 * mt, 128),
                                 coeffN[:, bs], start=(mt == 0), stop=False,
                                 skip_group_check=True)
            # scaled s0: one op per 128-feat half, 8 experts each
            sca = scp.tile([128, E * BT], F32R, tag="sc1a", name=f"sc1a{k}")
            nc.vector.tensor_mul(sca[:], _rep_ap(s0[:, 0:BT], E, BT), cb[:])
            scb = scp.tile([128, E * BT], F32R, tag="sc1b", name=f"sc1b{k}")
            nc.vector.tensor_mul(scb[:], _rep_ap(s0[:, BT:], E, BT), cb[:])
            for e in range(E):
                for half, sct in ((0, sca), (1, scb)):
                    kt = 2 * e + half
                    for mt in range(2):
                        nc.tensor.matmul(ps[:, BT * mt:BT * (mt + 1)],
                                         wsl("w1h", 0, 128, 256 * kt + 128 * mt, 128),
                                         sct[:, BT * e:BT * (e + 1)],
                                         start=False, stop=False,
                                         skip_group_check=True)
            # z-part last: gpsimd-built zs gets the whole expert phase to land
            for mt in range(2):
                for kt in range(2):
                    nc.tensor.matmul(ps[:, BT * mt:BT * (mt + 1)],
                                     wsl("w1z", 0, 128, 256 * kt + 128 * mt, 128),
                                     zs2[:, R * kt + BT * k:R * kt + BT * (k + 1)],
                                     start=False,
                                     stop=(mt == 1 and kt == 1),
                                     skip_group_check=True)
            s1 = s0p.tile([128, 2 * BT], F32R, tag="s1", name=f"s1_{k}")
            elu_combine(ps, s1)

            per2 = pm2.tile([128, BT], F32, tag="mm2", name=f"ps2_{k}")
            nc.tensor.matmul(per2[:], wsl("w2s", 0, 32, 0, 128), zrep[0:32, bs],
                             start=True, stop=False)
            nc.tensor.matmul(per2[:], wsl("w2s", 0, 128, 128, 128), s1[:, 0:BT],
                             start=False, stop=False)
            nc.tensor.matmul(per2[:], wsl("w2s", 0, 128, 256, 128), s1[:, BT:],
                             start=False, stop=True)
            mixed = er.tile([128, BT], F32R, tag="mx")
            nc.vector.scalar_tensor_tensor(mixed[:], per2[:], b2c[:], cbe[:],
                                           OP.add, OP.mult)
            op = po.tile([128, 2 * ACTD], F32, tag="op", name=f"op{k}")
            for jj in range(BT // 128):
                nc.tensor.matmul(op[:, ACTD * jj:ACTD * (jj + 1)],
                                 mixed[:, 128 * jj:128 * (jj + 1)],
                                 wsl("s2", 0, 128, 0, 16),
                                 start=(jj == 0), stop=(jj == 1),
                                 skip_group_check=True)
            ob = er.tile([128, 2 * ACTD], F32, tag="ob", name=f"ob{k}")
            nc.vector.tensor_copy(ob[:], op[:])
            # store chunk: out row 8p+r <- ob[p, 16jj..]
            nc.scalar.dma_start(
                AP(out_d.tensor, 2 * ACTD * k,
                   [[NLC * ACTD, 128], [1, 2 * ACTD]]),
                ob[:])

        # ---------------- schedule ----------------
        # Chunks 0/1's full chains run first and tight; chunks 2/3's gates
        # trail, interleaved between early layer phases.  The deferred wall
        # load lands during the gate phase.
        cbs = {}
        s0s = {}
        emit_stats(0)
        emit_cn(0); emit_cn(1)
        gate_a(0); gate_a(1)
        gate_b(0); gate_b(1)
        emit_stats(1)
        emit_cn(2); emit_cn(3)
        gate_c(0); gate_d(0)
        cbs[0] = emit_bcast(0)
        emit_wall_load()
        gate_c(1); gate_d(1)
        cbs[1] = emit_bcast(1)
        gate_a(2); gate_a(3)
        s0s[0] = emit_l0(0, cbs[0][0])
        gate_b(2); gate_b(3)
        s0s[1] = emit_l0(1, cbs[1][0])
        gate_c(2); gate_d(2)
        cbs[2] = emit_bcast(2)
        emit_l12(0, s0s[0], cbs[0][0], cbs[0][1])
        gate_c(3); gate_d(3)
        cbs[3] = emit_bcast(3)
        s0s[2] = emit_l0(2, cbs[2][0])
        emit_l12(1, s0s[1], cbs[1][0], cbs[1][1])
        s0s[3] = emit_l0(3, cbs[3][0])
        emit_l12(2, s0s[2], cbs[2][0], cbs[2][1])
        emit_l12(3, s0s[3], cbs[3][0], cbs[3][1])

    nc.compile()
    return nc


def _host_prep(inputs):
    f = lambda a: np.ascontiguousarray(np.asarray(a, dtype=np.float32))
    w0, b0 = f(inputs["w0"]), f(inputs["b0"])
    w1, b1 = f(inputs["w1"]), f(inputs["b1"])
    w2, b2 = f(inputs["w2"]), f(inputs["b2"])
    g0w, g0b = f(inputs["g0w"]), f(inputs["g0b"])
    g1w, g1b = f(inputs["g1w"]), f(inputs["g1b"])
    g2w, g2b = f(inputs["g2w"]), f(inputs["g2b"])
    ln_g, ln_b = f(inputs["ln_g"]), f(inputs["ln_b"])

    # fold LN affine into everything that consumes cn: cn = xhat*g + b
    w0c_f = w0[:, LATENT:, :] * ln_g[None, :, None]       # [E, CIN, HID]
    b0_f = b0 + np.einsum('i,eio->eo', ln_b, w0[:, LATENT:, :])
    g0c_f = g0w[LATENT:] * ln_g[:, None]                  # [CIN, GH]
    g0b_f = g0b + ln_b @ g0w[LATENT:]

    def ksb(wstk, nkt, m):   # [nkt*128, m] -> [128, nkt*m]
        return np.ascontiguousarray(
            wstk.reshape(nkt, 128, m).transpose(1, 0, 2).reshape(128, nkt * m))

    wall = np.zeros((128, NWALL), np.float32)
    wgate = np.zeros((128, NGATE), np.float32)
    def put(name, arr):
        if name in _GOFF:
            o = _GOFF[name]
            wgate[:arr.shape[0], o:o + arr.shape[1]] = arr
        else:
            o = _WOFF[name]
            wall[:arr.shape[0], o:o + arr.shape[1]] = arr

    put("w0z", ksb(w0[:, :LATENT, :].reshape(E * LATENT, HID), 2, HID))
    put("w0c", ksb(w0c_f.reshape(E * CIN, HID), 8, HID))
    put("w1z", ksb(w1[:, :LATENT, :].reshape(E * LATENT, HID), 2, HID))
    put("w1h", ksb(w1[:, LATENT:, :].reshape(E * HID, HID), 16, HID))
    w2stk = w2.transpose(1, 0, 2).reshape(INTER, E * ACTD)   # [288, 128]
    w2s = np.zeros((128, 384), np.float32)
    w2s[:32, 0:128] = w2stk[0:32]
    w2s[:, 128:256] = w2stk[32:160]
    w2s[:, 256:384] = w2stk[160:288]
    put("w2s", w2s)
    put("s2", np.tile(np.eye(ACTD, dtype=np.float32), (E, 1)))
    put("g0z", g0w[:LATENT])
    put("g0c", g0c_f)
    put("g1w", g1w)
    put("g2w", g2w)
    b1f = b1 - w1[:, LATENT:, :].sum(axis=1)
    put("b01", np.concatenate([b0_f, b1f], axis=1))
    put("on8", np.ones((8, 1), np.float32))
    put("onr", np.ones((1, 8), np.float32))
    put("i16", np.eye(128, dtype=np.float32))

    b2f = b2 - w2[:, LATENT:, :].sum(axis=1)                 # [8,16]
    consts = np.zeros((128, 7), np.float32)
    consts[:, 2] = LN_EPS
    consts[:, 3] = g0b_f
    consts[:, 4] = g1b - g1w.sum(0)
    consts[:, 5] = b2f.reshape(128)
    consts[:8, 6] = (g2b - g2w.sum(0))
    return {"wall": wall.astype(np.float16), "wgate": wgate.astype(np.float16),
            "consts": consts}


def make_in_maps(inputs):
    wmap = _host_prep(inputs)
    z = np.ascontiguousarray(np.asarray(inputs["z"], dtype=np.float32))
    c = np.ascontiguousarray(np.asarray(inputs["c"], dtype=np.float32))
    # on-chip batch order: i = 128*r + p  <->  original row b = 8p + r
    ii = np.arange(R)
    perm = 8 * (ii % 128) + ii // 128
    in_maps = []
    for i in range(N_CORES):
        m = dict(wmap)
        zsh = z[i * R:(i + 1) * R]
        m["zrep"] = np.ascontiguousarray(np.tile(zsh.T[:, perm], (4, 1))).astype(np.float16)
        csh = c[i * R:(i + 1) * R]
        # partition p <- rows 8p..8p+8 (contiguous 4KB lines)
        m["cperm"] = np.ascontiguousarray(csh.reshape(128, NLC * CIN))
        in_maps.append(m)
    return in_maps


def kernel(**inputs):
    if "nc" not in _CACHE:
        _CACHE["nc"] = _build_program()
    nc = _CACHE["nc"]
    in_maps = make_in_maps(inputs)
    res = bass_utils.run_bass_kernel_spmd(nc, in_maps, core_ids=list(range(N_CORES)))
    return np.concatenate([res.results[i]["out"] for i in range(N_CORES)], axis=0)


# revision 50
# speedup vs baseline: 1.2257x; 1.2257x over previous
"""Trainium2 Bass kernel for nn_MixedMlp (soft-mixture MoE MLP).

Math (per batch row b):
    cn = LayerNorm(c); x = [z, cn]
    coeff = softmax(gateMLP(x))                       # [E]
    l0 = elu(sum_e coeff_e (x @ w0_e + b0_e))
    l1 = elu(sum_e coeff_e ([z, l0] @ w1_e + b1_e))
    out = sum_e coeff_e ([z, l1] @ w2_e + b2_e)

Kernel strategy (8 cores, data-parallel over B=8192), v2 pipelined:
  * Activations feature-major ([features, batch]); every layer is one
    PSUM-accumulated GEMM contracting K = E*in over coeff-scaled inputs.
  * Batch split into 4 chunks of 256 rows; LN -> gate -> coeff broadcast
    -> expert layers -> store software-pipelined across chunks.
  * c ships both batch-major fp32 (for bn_stats) and feature-major fp16
    (cT); LN applies as cn = (cT - m_bc) * rstd_bc with mean/rstd
    DMA-broadcast along partitions — no PE transposes, no scalar affine.
    gamma/beta folded into w0c/g0c host-side.
  * Each layer's two 128-row output halves accumulate into ONE [128,512]
    PSUM bank tile, so elu exp/relu/combine run as single 512-wide ops.
  * Scaled inputs built as one DVE tensor_mul per (layer, feat-tile,
    chunk): stride-0 repeat of the activation x expert-major coeff
    broadcast (8 experts per instruction). GpSimd only does the small
    z-scaling; it is ~4x slower than DVE per element.
  * ELU: s = elu(x)+1 = relu(x) + min(exp(x),1); -1 folded into next
    bias host-side. exp/relu on Scalar, combine on DVE.
  * Layer 2 stacks per-expert outputs [(e,o), b], mixes by expanded
    coeff, selector-matmuls into [128,32] PSUM, DMA'd straight to DRAM.
  * coeff broadcasts via DMA replication from a DRAM staging buffer,
    per-chunk, spread across sync/gpsimd/scalar queues.
"""

import numpy as np
from contextlib import ExitStack

import concourse.bass as bass
import concourse.bacc as bacc
import concourse.tile as tile
import concourse.mybir as mybir
from concourse import bass_utils
from concourse.bass import AP

F32 = mybir.dt.float32
F32R = mybir.dt.float16  # full fp16 datapath
AF = mybir.ActivationFunctionType
OP = mybir.AluOpType

N_CORES = 8
B = 8192
R = B // N_CORES          # rows per core = 1024
LATENT, CIN, HID, ACTD, E, GH = 32, 128, 256, 16, 8, 128
IN0, INTER = LATENT + CIN, HID + LATENT
LN_EPS = 1e-5
BT = 256                  # batch chunk (pipeline granularity)
NCH = R // BT             # 4 chunks per core
NLC = R // 128            # 8 ln-chunks per core

# packed-weight column offsets (two packs: gate-critical, expert bulk)
_GCOLS = [("g0z", 128), ("g0c", 128), ("g1w", 128), ("g2w", 8),
          ("b01", 512), ("on8", 1), ("onr", 8), ("i16", 128)]
_WCOLS = [("w0z", 512), ("w0c", 2048), ("w1z", 512), ("w1h", 4096),
          ("w2s", 384), ("s2", 16)]
_GOFF, _WOFF = {}, {}
_o = 0
for _n, _c in _GCOLS:
    _GOFF[_n] = _o
    _o += _c
NGATE = _o
_o = 0
for _n, _c in _WCOLS:
    _WOFF[_n] = _o
    _o += _c
NWALL = _o

_CACHE = {}


def _rep_ap(t_ap, nrep, width):
    """AP reading t_ap[:, 0:width] repeated nrep times along a stride-0
    middle dim: shape [128, nrep, width]."""
    part = list(t_ap.ap[0])
    return AP(t_ap.tensor, t_ap.offset, [part, [0, nrep], [1, width]])


def _build_program():
    nc = bacc.Bacc("TRN2", target_bir_lowering=False, debug=False,
                   num_devices=N_CORES)

    zr_d = nc.dram_tensor("zrep", [128, R], F32R, kind="ExternalInput").ap()
    c_d = nc.dram_tensor("cperm", [128, NLC * CIN], F32, kind="ExternalInput").ap()
    wg_d = nc.dram_tensor("wgate", [128, NGATE], F32R, kind="ExternalInput").ap()
    wall_d = nc.dram_tensor("wall", [128, NWALL], F32R, kind="ExternalInput").ap()
    ck_d = nc.dram_tensor("consts", [128, 7], F32, kind="ExternalInput").ap()
    out_d = nc.dram_tensor("out", [R, ACTD], F32, kind="ExternalOutput").ap()

    with tile.TileContext(nc) as tc, ExitStack() as ctx:
        wp = ctx.enter_context(tc.tile_pool(name="wp", bufs=1))       # weights
        big = ctx.enter_context(tc.tile_pool(name="big", bufs=1))     # persistent
        sp = ctx.enter_context(tc.tile_pool(name="sp", bufs=4))       # small temps
        er = ctx.enter_context(tc.tile_pool(name="er", bufs=4))       # elu temps
        hp = ctx.enter_context(tc.tile_pool(name="hp", bufs=4))       # gate hiddens
        scp = ctx.enter_context(tc.tile_pool(name="scp", bufs=2))     # scaled inputs
        s0p = ctx.enter_context(tc.tile_pool(name="s0p", bufs=2))     # layer outs
        cbp = ctx.enter_context(tc.tile_pool(name="cbp", bufs=4))     # coeff bcasts
        pm = ctx.enter_context(tc.tile_pool(name="pm", bufs=2, space="PSUM"))
        pm2 = ctx.enter_context(tc.tile_pool(name="pm2", bufs=4, space="PSUM"))
        pt = ctx.enter_context(tc.tile_pool(name="pt", bufs=1, space="PSUM"))
        po = ctx.enter_context(tc.tile_pool(name="po", bufs=1, space="PSUM"))
        dstage = ctx.enter_context(tc.tile_pool(name="dstage", bufs=1, space="DRAM"))

        # ---------------- bulk loads ----------------
        # cperm: partition p holds rows 8p..8p+8 (stats layout).  ctr/zrep:
        # feature-major, on-chip batch order i = 128*r + p <-> row 8p + r.
        # priority order: LN inputs + gate weights first; the big expert
        # weight pack (wall) is deferred until after the gate emission so
        # its 2MB doesn't clog the DMA engines ahead of the critical path.
        ctall = big.tile([128, NLC * CIN], F32)
        ckt = wp.tile([128, 7], F32)
        nc.sync.dma_start(ckt[:], ck_d[:])
        for hh in range(2):
            cs = slice(2 * CIN * hh, 2 * CIN * (hh + 1))
            nc.sync.dma_start(ctall[:, cs], c_d[:, cs])
        wgate = wp.tile([128, NGATE], F32R)
        nc.scalar.dma_start(wgate[:], wg_d[:], max_dma_last_dim=4096)
        zrep = big.tile([128, R], F32R)
        nc.scalar.dma_start(zrep[:], zr_d[:])
        for hh in range(2, 4):
            cs = slice(2 * CIN * hh, 2 * CIN * (hh + 1))
            nc.sync.dma_start(ctall[:, cs], c_d[:, cs])
        wall = wp.tile([128, NWALL], F32R)

        def emit_wall_load():
            h2 = NWALL // 2
            nc.scalar.dma_start(wall[:, 0:h2], wall_d[:, 0:h2],
                                max_dma_last_dim=4096)
            nc.scalar.dma_start(wall[:, h2:], wall_d[:, h2:],
                                max_dma_last_dim=4096)

        def wsl(name, p0, pn, c0, cn_):
            if name in _GOFF:
                o = _GOFF[name]
                return wgate[p0:p0 + pn, o + c0:o + c0 + cn_]
            o = _WOFF[name]
            return wall[p0:p0 + pn, o + c0:o + c0 + cn_]
        epsc = ckt[:, 2:3]
        g0b, g1b, b2c = ckt[:, 3:4], ckt[:, 4:5], ckt[:, 5:6]
        g2b = ckt[0:8, 6:7]

        # ---------------- persistent tiles ----------------
        cnT = big.tile([128, R], F32R)     # LayerNormed c (affine folded)
        zs2 = big.tile([128, 2 * R], F32R)  # coeff-scaled z, 2 quad-groups
        eL = big.tile([8, R], F32R)        # exp(gate logits)
        coeffN = big.tile([8, R], F32R)    # softmax coeffs
        mv8 = big.tile([128, 2 * NLC], F32)
        rstd8 = big.tile([128, NLC], F32)

        cstage = dstage.tile([8, R], F32R)
        ctens = cstage.tensor

        # ------- LN stats in two batches; rstd via Ln+Exp each -------
        lnv8 = big.tile([128, NLC], F32)

        def emit_stats(b):
            for j in range(4 * b, 4 * b + 4):
                ct = ctall[:, CIN * j:CIN * (j + 1)]
                stats = sp.tile([128, 6], F32, tag="st")
                nc.vector.bn_stats(stats[:], ct[:])
                nc.vector.bn_aggr(mv8[:, 2 * j:2 * j + 2], stats[:])
            var4 = AP(mv8[:].tensor, mv8[:].offset + 8 * b + 1,
                      [list(mv8[:].ap[0]), [2, 4]])
            nc.scalar.activation(lnv8[:, 4 * b:4 * b + 4], var4, AF.Ln,
                                 bias=epsc[:])
            nc.scalar.activation(rstd8[:, 4 * b:4 * b + 4],
                                 lnv8[:, 4 * b:4 * b + 4], AF.Exp, scale=-0.5)

        # ---------------- per-chunk emitters ----------------
        # LN tail on DVE: scale via tensor_scalar, evac via copy.
        def emit_cn(k):
            for j in (2 * k, 2 * k + 1):
                js = slice(CIN * j, CIN * (j + 1))
                y = sp.tile([128, CIN], F32R, tag="y")
                nc.vector.tensor_scalar(y[:], ctall[:, js],
                                        mv8[:, 2 * j:2 * j + 1],
                                        rstd8[:, j:j + 1],
                                        OP.subtract, OP.mult)
                yT = pt.tile([128, CIN], F32R, tag="tp")
                nc.tensor.transpose(yT[:], y[:], wsl("i16", 0, 128, 0, 128))
                nc.vector.tensor_copy(cnT[:, js], yT[:])

        # gate at 256-wide, 4 chains staged so they pipeline each other's
        # cross-engine latency
        _g = [dict() for _ in range(NCH)]

        def gate_a(k):
            bs = slice(BT * k, BT * (k + 1))
            pre0 = pm2.tile([128, BT], F32, tag="mm2", name=f"g0_{k}")
            nc.tensor.matmul(pre0[:], wsl("g0z", 0, 32, 0, 128), zrep[0:32, bs],
                             start=True, stop=False)
            nc.tensor.matmul(pre0[:], wsl("g0c", 0, 128, 0, 128), cnT[:, bs],
                             start=False, stop=True)
            e0 = er.tile([128, BT], F32R, tag="eg", name=f"e0_{k}")
            nc.scalar.activation(e0[:], pre0[:], AF.Exp, bias=g0b[:])
            r0 = er.tile([128, BT], F32R, tag="rg", name=f"r0_{k}")
            nc.vector.tensor_scalar(r0[:], pre0[:], g0b[:], 0.0,
                                    OP.add, OP.max)
            h0 = hp.tile([128, BT], F32R, tag="h0", name=f"h0_{k}")
            nc.vector.scalar_tensor_tensor(h0[:], e0[:], 1.0, r0[:],
                                           OP.min, OP.add)
            _g[k]["h0"] = h0

        def gate_b(k):
            pre1 = pm2.tile([128, BT], F32, tag="mm2", name=f"g1_{k}")
            nc.tensor.matmul(pre1[:], wsl("g1w", 0, 128, 0, 128),
                             _g[k]["h0"][:], start=True, stop=True)
            e1 = er.tile([128, BT], F32R, tag="eg", name=f"e1_{k}")
            nc.scalar.activation(e1[:], pre1[:], AF.Exp, bias=g1b[:])
            r1 = er.tile([128, BT], F32R, tag="rg", name=f"r1_{k}")
            nc.vector.tensor_scalar(r1[:], pre1[:], g1b[:], 0.0,
                                    OP.add, OP.max)
            h1 = hp.tile([128, BT], F32R, tag="h1", name=f"h1_{k}")
            nc.vector.scalar_tensor_tensor(h1[:], e1[:], 1.0, r1[:],
                                           OP.min, OP.add)
            _g[k]["h1"] = h1

        def gate_c(k):
            bs = slice(BT * k, BT * (k + 1))
            pre2 = pm2.tile([8, BT], F32, tag="mm2", name=f"g2_{k}")
            nc.tensor.matmul(pre2[:], wsl("g2w", 0, 128, 0, 8),
                             _g[k]["h1"][:], start=True, stop=True)
            nc.scalar.activation(eL[:, bs], pre2[:], AF.Exp, bias=g2b[:])
            sume = pm2.tile([1, BT], F32, tag="mm2", name=f"gs_{k}")
            nc.tensor.matmul(sume[:], wsl("on8", 0, 8, 0, 1), eL[:, bs],
                             start=True, stop=True)
            rsum = sp.tile([1, BT], F32, tag="rsm", name=f"rsum{k}")
            nc.vector.reciprocal_approx_fast(rsum[:], sume[:])
            rsr = sp.tile([1, BT], F32R, tag="rsr", name=f"rsr{k}")
            nc.vector.tensor_copy(rsr[:], rsum[:])
            _g[k]["rsr"] = rsr

        def gate_d(k):
            bs = slice(BT * k, BT * (k + 1))
            rbc = pm2.tile([8, BT], F32, tag="mm2", name=f"gr_{k}")
            nc.tensor.matmul(rbc[:], wsl("onr", 0, 1, 0, 8), _g[k]["rsr"][:],
                             start=True, stop=True)
            nc.vector.tensor_mul(coeffN[:, bs], eL[:, bs], rbc[:])

        def emit_bcast(k):
            bs = slice(BT * k, BT * (k + 1))
            o = BT * k
            nc.sync.dma_start(cstage[:, bs], coeffN[:, bs])
            cb = cbp.tile([128, E * BT], F32R, tag="cb", name=f"cb{k}")
            for eh, q in ((0, nc.sync), (1, nc.gpsimd), (2, nc.scalar),
                          (3, nc.sync)):
                q.dma_start(
                    AP(cb.tensor, cb[:].offset + 2 * eh * BT,
                       [[E * BT, 128], [BT, 2], [1, BT]]),
                    AP(ctens, 2 * eh * R + o, [[0, 128], [R, 2], [1, BT]]))
            cbz = cbp.tile([128, 2 * BT], F32R, tag="cbz", name=f"cbz{k}")
            for q in range(2):
                nc.sync.dma_start(
                    cbz[:, BT * q:BT * (q + 1)],
                    AP(ctens, 4 * q * R + o, [[R, 4], [0, 32], [1, BT]]))
            cbe = cbp.tile([128, BT], F32R, tag="cbe", name=f"cbe{k}")
            nc.gpsimd.dma_start(
                cbe[:], AP(ctens, o, [[R, 8], [0, 16], [1, BT]]))
            # coeff-scaled z (both quad groups in one gpsimd op)
            dst = AP(zs2.tensor, zs2[:].offset + o,
                     [[2 * R, 128], [R, 2], [1, BT]])
            src0 = AP(zrep.tensor, zrep[:].offset + o,
                      [[R, 128], [0, 2], [1, BT]])
            nc.gpsimd.tensor_mul(dst, src0, cbz[:])
            return cb, cbe

        def elu_combine(ps, dst):
            ee = er.tile([128, 2 * BT], F32R, tag="e")
            nc.scalar.activation(ee[:], ps[:], AF.Exp)
            rr = er.tile([128, 2 * BT], F32R, tag="r")
            nc.scalar.activation(rr[:], ps[:], AF.Relu)
            nc.vector.scalar_tensor_tensor(dst[:], ee[:], 1.0, rr[:],
                                           OP.min, OP.add)

        def emit_l0(k, cb):
            bs = slice(BT * k, BT * (k + 1))
            # one [128,512] bank tile for both output halves: start=True only
            # on the very first matmul (it bank-clears); later slices
            # auto-zero on first write via the pending-zero region.
            # z-part matmuls go LAST so the gpsimd-built zs has time to land.
            ps = pm.tile([128, 2 * BT], F32, tag="mm", name=f"ps0_{k}")
            for mt in range(2):
                nc.tensor.matmul(ps[:, BT * mt:BT * (mt + 1)],
                                 wsl("b01", 0, 8, 128 * mt, 128),
                                 coeffN[:, bs], start=(mt == 0), stop=False,
                                 skip_group_check=True)
            sc0 = scp.tile([128, E * BT], F32R, tag="sc0", name=f"sc0_{k}")
            nc.vector.tensor_mul(sc0[:], _rep_ap(cnT[:, bs], E, BT), cb[:])
            for e in range(E):
                for mt in range(2):
                    nc.tensor.matmul(ps[:, BT * mt:BT * (mt + 1)],
                                     wsl("w0c", 0, 128, 256 * e + 128 * mt, 128),
                                     sc0[:, BT * e:BT * (e + 1)],
                                     start=False, stop=False,
                                     skip_group_check=True)
            for mt in range(2):
                for kt in range(2):
                    nc.tensor.matmul(ps[:, BT * mt:BT * (mt + 1)],
                                     wsl("w0z", 0, 128, 256 * kt + 128 * mt, 128),
                                     zs2[:, R * kt + BT * k:R * kt + BT * (k + 1)],
                                     start=False,
                                     stop=(mt == 1 and kt == 1),
                                     skip_group_check=True)
            s0 = s0p.tile([128, 2 * BT], F32R, tag="s0", name=f"s0_{k}")
            elu_combine(ps, s0)
            return s0

        def emit_l12(k, s0, cb, cbe):
            bs = slice(BT * k, BT * (k + 1))
            ps = pm.tile([128, 2 * BT], F32, tag="mm", name=f"ps1_{k}")
            for mt in range(2):
                nc.tensor.matmul(ps[:, BT * mt:BT * (mt + 1)],
                                 wsl("b01", 0, 8, 256 + 128 * mt, 128),
                                 coeffN[:, bs], start=(mt == 0), stop=False,
                                 skip_group_check=True)
            # scaled s0: one op per 128-feat half, 8 experts each
            sca = scp.tile([128, E * BT], F32R, tag="sc1a", name=f"sc1a{k}")
            nc.vector.tensor_mul(sca[:], _rep_ap(s0[:, 0:BT], E, BT), cb[:])
            scb = scp.tile([128, E * BT], F32R, tag="sc1b", name=f"sc1b{k}")
            nc.vector.tensor_mul(scb[:], _rep_ap(s0[:, BT:], E, BT), cb[:])
            for e in range(E):
                for half, sct in ((0, sca), (1, scb)):
                    kt = 2 * e + half
                    for mt in range(2):
                        nc.tensor.matmul(ps[:, BT * mt:BT * (mt + 1)],
                                         wsl("w1h", 0, 128, 256 * kt + 128 * mt, 128),
                                         sct[:, BT * e:BT * (e + 1)],
                                         start=False, stop=False,
                                         skip_group_check=True)
            # z-part last: gpsimd-built zs gets the whole expert phase to land
            for mt in range(2):
                for kt in range(2):
                    nc.tensor.matmul(ps[:, BT * mt:BT * (mt + 1)],
                                     wsl("w1z", 0, 128, 256 * kt + 128 * mt, 128),
                                     zs2[:, R * kt + BT * k:R * kt + BT * (k + 1)],
                                     start=False,
                                     stop=(mt == 1 and kt == 1),
                                     skip_group_check=True)
            s1 = s0p.tile([128, 2 * BT], F32R, tag="s1", name=f"s1_{k}")
            elu_combine(ps, s1)

            per2 = pm2.tile([128, BT], F32, tag="mm2", name=f"ps2_{k}")
            nc.tensor.matmul(per2[:], wsl("w2s", 0, 32, 0, 128), zrep[0:32, bs],
                             start=True, stop=False)
            nc.tensor.matmul(per2[:], wsl("w2s", 0, 128, 128, 128), s1[:, 0:BT],
                             start=False, stop=False)
            nc.tensor.matmul(per2[:], wsl("w2s", 0, 128, 256, 128), s1[:, BT:],
                             start=False, stop=True)
            mixed = er.tile([128, BT], F32R, tag="mx")
            nc.vector.scalar_tensor_tensor(mixed[:], per2[:], b2c[:], cbe[:],
                                           OP.add, OP.mult)
            op = po.tile([128, 2 * ACTD], F32, tag="op", name=f"op{k}")
            for jj in range(BT // 128):
                nc.tensor.matmul(op[:, ACTD * jj:ACTD * (jj + 1)],
                                 mixed[:, 128 * jj:128 * (jj + 1)],
                                 wsl("s2", 0, 128, 0, 16),
                                 start=(jj == 0), stop=(jj == 1),
                                 skip_group_check=True)
            ob = er.tile([128, 2 * ACTD], F32, tag="ob", name=f"ob{k}")
            nc.vector.tensor_copy(ob[:], op[:])
            # store chunk: out row 8p+r <- ob[p, 16jj..]
            nc.scalar.dma_start(
                AP(out_d.tensor, 2 * ACTD * k,
                   [[NLC * ACTD, 128], [1, 2 * ACTD]]),
                ob[:])

        # ---------------- schedule ----------------
        # Chunks 0/1's full chains run first and tight; chunks 2/3's gates
        # trail, interleaved between early layer phases.  The deferred wall
        # load lands during the gate phase.
        cbs = {}
        s0s = {}
        emit_stats(0)
        emit_cn(0); emit_cn(1)
        gate_a(0); gate_a(1)
        gate_b(0); gate_b(1)
        emit_stats(1)
        emit_cn(2); emit_cn(3)
        gate_a(2); gate_a(3)
        gate_c(0); gate_d(0)
        cbs[0] = emit_bcast(0)
        emit_wall_load()
        gate_b(2); gate_b(3)
        gate_c(1); gate_d(1)
        cbs[1] = emit_bcast(1)
        gate_c(2); gate_d(2)
        cbs[2] = emit_bcast(2)
        gate_c(3); gate_d(3)
        cbs[3] = emit_bcast(3)
        s0s[0] = emit_l0(0, cbs[0][0])
        s0s[1] = emit_l0(1, cbs[1][0])
        emit_l12(0, s0s[0], cbs[0][0], cbs[0][1])
        s0s[2] = emit_l0(2, cbs[2][0])
        emit_l12(1, s0s[1], cbs[1][0], cbs[1][1])
        s0s[3] = emit_l0(3, cbs[3][0])
        emit_l12(2, s0s[2], cbs[2][0], cbs[2][1])
        emit_l12(3, s0s[3], cbs[3][0], cbs[3][1])

    nc.compile()
    return nc


def _host_prep(inputs):
    f = lambda a: np.ascontiguousarray(np.asarray(a, dtype=np.float32))
    w0, b0 = f(inputs["w0"]), f(inputs["b0"])
    w1, b1 = f(inputs["w1"]), f(inputs["b1"])
    w2, b2 = f(inputs["w2"]), f(inputs["b2"])
    g0w, g0b = f(inputs["g0w"]), f(inputs["g0b"])
    g1w, g1b = f(inputs["g1w"]), f(inputs["g1b"])
    g2w, g2b = f(inputs["g2w"]), f(inputs["g2b"])
    ln_g, ln_b = f(inputs["ln_g"]), f(inputs["ln_b"])

    # fold LN affine into everything that consumes cn: cn = xhat*g + b
    w0c_f = w0[:, LATENT:, :] * ln_g[None, :, None]       # [E, CIN, HID]
    b0_f = b0 + np.einsum('i,eio->eo', ln_b, w0[:, LATENT:, :])
    g0c_f = g0w[LATENT:] * ln_g[:, None]                  # [CIN, GH]
    g0b_f = g0b + ln_b @ g0w[LATENT:]

    def ksb(wstk, nkt, m):   # [nkt*128, m] -> [128, nkt*m]
        return np.ascontiguousarray(
            wstk.reshape(nkt, 128, m).transpose(1, 0, 2).reshape(128, nkt * m))

    wall = np.zeros((128, NWALL), np.float32)
    wgate = np.zeros((128, NGATE), np.float32)
    def put(name, arr):
        if name in _GOFF:
            o = _GOFF[name]
            wgate[:arr.shape[0], o:o + arr.shape[1]] = arr
        else:
            o = _WOFF[name]
            wall[:arr.shape[0], o:o + arr.shape[1]] = arr

    put("w0z", ksb(w0[:, :LATENT, :].reshape(E * LATENT, HID), 2, HID))
    put("w0c", ksb(w0c_f.reshape(E * CIN, HID), 8, HID))
    put("w1z", ksb(w1[:, :LATENT, :].reshape(E * LATENT, HID), 2, HID))
    put("w1h", ksb(w1[:, LATENT:, :].reshape(E * HID, HID), 16, HID))
    w2stk = w2.transpose(1, 0, 2).reshape(INTER, E * ACTD)   # [288, 128]
    w2s = np.zeros((128, 384), np.float32)
    w2s[:32, 0:128] = w2stk[0:32]
    w2s[:, 128:256] = w2stk[32:160]
    w2s[:, 256:384] = w2stk[160:288]
    put("w2s", w2s)
    put("s2", np.tile(np.eye(ACTD, dtype=np.float32), (E, 1)))
    put("g0z", g0w[:LATENT])
    put("g0c", g0c_f)
    put("g1w", g1w)
    put("g2w", g2w)
    b1f = b1 - w1[:, LATENT:, :].sum(axis=1)
    put("b01", np.concatenate([b0_f, b1f], axis=1))
    put("on8", np.ones((8, 1), np.float32))
    put("onr", np.ones((1, 8), np.float32))
    put("i16", np.eye(128, dtype=np.float32))

    b2f = b2 - w2[:, LATENT:, :].sum(axis=1)                 # [8,16]
    consts = np.zeros((128, 7), np.float32)
    consts[:, 2] = LN_EPS
    consts[:, 3] = g0b_f
    consts[:, 4] = g1b - g1w.sum(0)
    consts[:, 5] = b2f.reshape(128)
    consts[:8, 6] = (g2b - g2w.sum(0))
    return {"wall": wall.astype(np.float16), "wgate": wgate.astype(np.float16),
            "consts": consts}


def make_in_maps(inputs):
    wmap = _host_prep(inputs)
    z = np.ascontiguousarray(np.asarray(inputs["z"], dtype=np.float32))
    c = np.ascontiguousarray(np.asarray(inputs["c"], dtype=np.float32))
    # on-chip batch order: i = 128*r + p  <->  original row b = 8p + r
    ii = np.arange(R)
    perm = 8 * (ii % 128) + ii // 128
    in_maps = []
    for i in range(N_CORES):
        m = dict(wmap)
        zsh = z[i * R:(i + 1) * R]
        m["zrep"] = np.ascontiguousarray(np.tile(zsh.T[:, perm], (4, 1))).astype(np.float16)
        csh = c[i * R:(i + 1) * R]
        # partition p <- rows 8p..8p+8 (contiguous 4KB lines)
        m["cperm"] = np.ascontiguousarray(csh.reshape(128, NLC * CIN))
        in_maps.append(m)
    return in_maps


def kernel(**inputs):
    if "nc" not in _CACHE:
        _CACHE["nc"] = _build_program()
    nc = _CACHE["nc"]
    in_maps = make_in_maps(inputs)
    res = bass_utils.run_bass_kernel_spmd(nc, in_maps, core_ids=list(range(N_CORES)))
    return np.concatenate([res.results[i]["out"] for i in range(N_CORES)], axis=0)
